# revision 1
# baseline (speedup 1.0000x reference)
"""TRN2 Bass kernel for DenseDilatedKnnGraph (B=4, C=64, N=4096, k=9, dilation=2).

Algorithm
---------
reference: xt (B,N,C); dist(i,j) = |xi|^2 - 2<xi,xj> + |xj|^2; nn_idx = top-18
of -dist per row (stable, lowest-index tie-break); output nn_idx[..., ::2] plus
a center-index row -> (2, B, N, 9) int32.

Per-row ordering of -dist is identical to the ordering of
    s_ij = 2<xi,xj> - |xj|^2
(the |xi|^2 term is constant per row), and s has better relative precision.

Device (per core, SPMD over 8 cores; core = (batch, query-half)):
  - matmul: s = lhsT.T @ rhsT with an augmented contraction dim of 65:
      lhsT[k,m] = 2*x[q0+m,k] (k<64), 1.0 (k=64)       (65 x 2048)
      rhsT[k,j] = x[j,k]      (k<64), -|xj|^2 (k=64)   (65 x 4096)
    fp32, PSUM accumulate; 128-query tiles, 512-wide PSUM chunks.
  - PSUM -> SBUF copy on the scalar engine.
  - DVE top-k: per 512-group max8 (values) + max_index (local indices,
    first-occurrence = lowest-index tie-break, matching jax.lax.top_k), then
    merge the 64 group-candidates with max8+match_replace rounds into the
    row's top-24 ranked values.
  - DMA out: ranked values T (128x24), group-candidate values U (128x64),
    local indices L (128x64).

Host: associates ranked values T with (U, L) pairs in slot order (slot order ==
global index order for equal values, preserving the stable tie-break), which
yields the top-18 global indices per row. Rows where a single 512-group
contributed 8 members to the top-18 (its 9th member could have been lost) or
where a value match fails are recomputed exactly on the host (rare: ~tens of
rows out of 16384).
"""

import numpy as np

import concourse.bacc as bacc
import concourse.mybir as mybir
import concourse.tile as tile
from concourse.bass_utils import run_bass_kernel_spmd

# Problem constants (hardcoded per harness contract).
B = 4
C = 64
N = 4096
K = 9
DILATION = 2
K_EFF = K * DILATION      # 18
P = 128                   # partitions / queries per tile
KAUG = C + 1              # augmented contraction dim
GROUP = 512               # candidates per DVE max8 group
NG = N // GROUP           # 8 groups
UW = NG * 8               # 64 group-candidates per row
TW = 24                   # ranked values extracted per row (3 rounds of 8)
N_CORES = 8
QROWS = (B * N) // N_CORES          # 2048 query rows per core
N_TILES = QROWS // P                # 16 tiles per core
NEG = -3.0e38


def _build_program(n_tiles=N_TILES):
    nc = bacc.Bacc(
        "TRN2", target_bir_lowering=False, debug=False, enable_asserts=False
    )
    f32 = mybir.dt.float32
    u32 = mybir.dt.uint32
    lhsT = nc.dram_tensor("lhsT", (KAUG, n_tiles * P), f32, kind="ExternalInput")
    rhsT = nc.dram_tensor("rhsT", (KAUG, N), f32, kind="ExternalInput")
    t_out = nc.dram_tensor("t_out", (n_tiles * P, TW), f32, kind="ExternalOutput")
    u_out = nc.dram_tensor("u_out", (n_tiles * P, UW), f32, kind="ExternalOutput")
    l_out = nc.dram_tensor("l_out", (n_tiles * P, UW), u32, kind="ExternalOutput")
    lhsT_ap, rhsT_ap = lhsT.ap(), rhsT.ap()
    t_ap, u_ap, l_ap = t_out.ap(), u_out.ap(), l_out.ap()

    with tile.TileContext(nc) as tc:
        with (
            tc.tile_pool(name="const", bufs=1) as cpool,
            tc.tile_pool(name="psum", bufs=2, space="PSUM") as ppool,
            tc.tile_pool(name="work", bufs=3) as wpool,
            tc.tile_pool(name="outp", bufs=3) as opool,
        ):
            rhs_sb = cpool.tile([KAUG, N], f32)
            for j in range(0, N, 512):
                nc.sync.dma_start(rhs_sb[:, j : j + 512], rhsT_ap[:, j : j + 512])
            lhs_sb = cpool.tile([KAUG, n_tiles * P], f32)
            for j in range(0, n_tiles * P, 512):
                w = min(512, n_tiles * P - j)
                nc.sync.dma_start(lhs_sb[:, j : j + w], lhsT_ap[:, j : j + w])

            for t in range(n_tiles):
                ssb = wpool.tile([P, N], f32, tag="ssb")
                for h in range(2):
                    ps = ppool.tile([P, N // 2], f32, tag="ps")
                    for j in range(4):
                        nc.tensor.matmul(
                            ps[:, j * 512 : (j + 1) * 512],
                            lhs_sb[:, t * P : (t + 1) * P],
                            rhs_sb[:, h * (N // 2) + j * 512 : h * (N // 2) + (j + 1) * 512],
                            start=True,
                            stop=True,
                        )
                    nc.scalar.copy(ssb[:, h * (N // 2) : (h + 1) * (N // 2)], ps[:, :])

                u = opool.tile([P, UW], f32, tag="u")
                l = opool.tile([P, UW], u32, tag="l")
                tt = opool.tile([P, TW], f32, tag="t")
                uscr = wpool.tile([P, UW], f32, tag="uscr")
                for g in range(NG):
                    nc.vector.max(
                        out=u[:, g * 8 : (g + 1) * 8],
                        in_=ssb[:, g * GROUP : (g + 1) * GROUP],
                    )
                for g in range(NG):
                    nc.vector.max_index(
                        out=l[:, g * 8 : (g + 1) * 8],
                        in_max=u[:, g * 8 : (g + 1) * 8],
                        in_values=ssb[:, g * GROUP : (g + 1) * GROUP],
                    )
                nc.vector.max(out=tt[:, 0:8], in_=u)
                nc.vector.match_replace(uscr, tt[:, 0:8], u, NEG)
                nc.vector.max(out=tt[:, 8:16], in_=uscr)
                nc.vector.match_replace(uscr, tt[:, 8:16], uscr, NEG)
                nc.vector.max(out=tt[:, 16:24], in_=uscr)

                rs = slice(t * P, (t + 1) * P)
                nc.sync.dma_start(t_ap[rs, :], tt[:])
                nc.sync.dma_start(u_ap[rs, :], u[:])
                nc.sync.dma_start(l_ap[rs, :], l[:])
    nc.compile()
    return nc


def _prep_core_inputs(X, core):
    """X: (B, N, C) fp32. Returns {'lhsT', 'rhsT'} for one core."""
    b, h = divmod(core, N_CORES // B)
    Xb = X[b]
    xsq = np.sum(Xb * Xb, axis=1, dtype=np.float32)
    rhsT = np.empty((KAUG, N), np.float32)
    rhsT[:C] = Xb.T
    rhsT[C] = -xsq
    Q = Xb[h * QROWS : (h + 1) * QROWS]
    lhsT = np.empty((KAUG, QROWS), np.float32)
    lhsT[:C] = (2.0 * Q).T
    lhsT[C] = 1.0
    return {"lhsT": np.ascontiguousarray(lhsT), "rhsT": np.ascontiguousarray(rhsT)}


def _zip_ranks(T, U, L):
    """Associate ranked values T (R,24) with slots (U values, L local idx)
    in slot order. Returns (idx (R,18) int64, bad-row mask (R,))."""
    R = T.shape[0]
    g_of_slot = (np.arange(UW, dtype=np.int64) // 8) * GROUP
    Gidx = L.astype(np.int64) + g_of_slot[None, :]
    used = np.zeros((R, UW), bool)
    out = np.zeros((R, K_EFF), np.int64)
    bad = np.zeros(R, bool)
    rows = np.arange(R)
    for k in range(K_EFF):
        m = (U == T[:, k : k + 1]) & ~used
        s = np.argmax(m, axis=1)
        ok = m[rows, s]
        bad |= ~ok
        out[:, k] = Gidx[rows, s]
        used[rows, s] = True
    # hazard: a group whose full top-8 landed in the top-18 may have lost a
    # 9th member that belongs there
    grp_used = used.reshape(R, NG, 8).sum(axis=2)
    bad |= (grp_used >= 8).any(axis=1)
    return out, bad


def _host_topk_row(Xb, xsq, q):
    s = (2.0 * Xb[q]) @ Xb.T
    s = (s - xsq).astype(np.float32)
    order = np.argsort(-s, kind="stable")
    return order[:K_EFF]


_NC_CACHE = {}


def kernel(x: np.ndarray) -> np.ndarray:
    x = np.asarray(x)
    assert x.shape == (B, C, N, 1), x.shape
    X = np.ascontiguousarray(np.transpose(x[..., 0], (0, 2, 1)))  # (B, N, C)

    if N_TILES not in _NC_CACHE:
        _NC_CACHE[N_TILES] = _build_program(N_TILES)
    nc = _NC_CACHE[N_TILES]

    in_maps = [_prep_core_inputs(X, c) for c in range(N_CORES)]
    res = run_bass_kernel_spmd(nc, in_maps, core_ids=list(range(N_CORES)))

    nn_idx = np.empty((B, N, K_EFF), np.int64)
    for core in range(N_CORES):
        b, h = divmod(core, N_CORES // B)
        r = res.results[core]
        idx, bad = _zip_ranks(r["t_out"], r["u_out"], r["l_out"])
        if bad.any():
            Xb = X[b]
            xsq = np.sum(Xb * Xb, axis=1, dtype=np.float32)
            for rr in np.nonzero(bad)[0]:
                idx[rr] = _host_topk_row(Xb, xsq, h * QROWS + rr)
        nn_idx[b, h * QROWS : (h + 1) * QROWS] = idx

    nn_dil = nn_idx[:, :, ::DILATION]                       # (B, N, 9)
    center = np.broadcast_to(np.arange(N)[None, :, None], nn_dil.shape)
    out = np.stack((nn_dil, center), axis=0).astype(np.int32)
    return out


# revision 2
# speedup vs baseline: 1.0179x; 1.0179x over previous
"""TRN2 Bass kernel for DenseDilatedKnnGraph (B=4, C=64, N=4096, k=9, dilation=2).

Algorithm
---------
reference: xt (B,N,C); dist(i,j) = |xi|^2 - 2<xi,xj> + |xj|^2; nn_idx = top-18
of -dist per row (stable, lowest-index tie-break); output nn_idx[..., ::2] plus
a center-index row -> (2, B, N, 9) int32.

Per-row ordering of -dist is identical to the ordering of
    s_ij = 2<xi,xj> - |xj|^2
(the |xi|^2 term is constant per row), and s has better relative precision.

Device (per core, SPMD over 8 cores; core = (batch, query-half)):
  - s computed via 3 fp16-speed matmuls (hi/lo split of fp32, error ~1e-6,
    ~4x cheaper than native fp32 matmul on the PE):
      s = qh@ch + (s1+s2+s3) + qh@cl + ql@ch
    where qh/ql = fp16 split of 2x (queries), ch/cl = fp16 split of x
    (candidates), s1..s3 = 3-level fp16 split of -|xj|^2 carried on three
    extra contraction rows of the first matmul. PSUM fp32 accumulate,
    128-query tiles, 512-wide PSUM chunks.
  - PSUM -> SBUF copy on the scalar engine.
  - DVE top-k: per GROUP-wide group max8 (values) + max_index (local indices,
    first-occurrence = lowest-index tie-break, matching jax.lax.top_k), then
    merge the group-candidates with max8+match_replace rounds into the
    row's top-24 ranked values.
  - DMA out: ranked values T (128x24), group-candidate values U, local
    indices L.

Host: associates ranked values T with (U, L) pairs in slot order (slot order ==
global index order for equal values, preserving the stable tie-break), which
yields the top-18 global indices per row. Rows where a single group
contributed 8 members to the top-18 (its 9th member could have been lost) or
where a value match fails are recomputed exactly on the host.
"""

import numpy as np

import concourse.bacc as bacc
import concourse.mybir as mybir
import concourse.tile as tile
from concourse.bass_utils import run_bass_kernel_spmd

# Problem constants (hardcoded per harness contract).
B = 4
C = 64
N = 4096
K = 9
DILATION = 2
K_EFF = K * DILATION      # 18
P = 128                   # partitions / queries per tile
KAUG = C + 3              # contraction dim of matmul 1 (64 + 3 bias rows)
GROUP = 512               # candidates per DVE max8 group
NG = N // GROUP
UW = NG * 8               # group-candidates per row
TW = 24                   # ranked values extracted per row (3 rounds of 8)
N_CORES = 8
QROWS = (B * N) // N_CORES          # 2048 query rows per core
N_TILES = QROWS // P                # 16 tiles per core
NEG = -3.0e38


def _build_program(n_tiles=N_TILES):
    nc = bacc.Bacc(
        "TRN2", target_bir_lowering=False, debug=False, enable_asserts=False
    )
    f32 = mybir.dt.float32
    f16 = mybir.dt.float16
    u32 = mybir.dt.uint32
    nq = n_tiles * P
    lhs_h = nc.dram_tensor("lhs_h", (KAUG, nq), f16, kind="ExternalInput")
    lhs_l = nc.dram_tensor("lhs_l", (C, nq), f16, kind="ExternalInput")
    rhs_h = nc.dram_tensor("rhs_h", (KAUG, N), f16, kind="ExternalInput")
    rhs_l = nc.dram_tensor("rhs_l", (C, N), f16, kind="ExternalInput")
    t_out = nc.dram_tensor("t_out", (nq, TW), f32, kind="ExternalOutput")
    u_out = nc.dram_tensor("u_out", (nq, UW), f32, kind="ExternalOutput")
    l_out = nc.dram_tensor("l_out", (nq, UW), u32, kind="ExternalOutput")
    lhs_h_ap, lhs_l_ap = lhs_h.ap(), lhs_l.ap()
    rhs_h_ap, rhs_l_ap = rhs_h.ap(), rhs_l.ap()
    t_ap, u_ap, l_ap = t_out.ap(), u_out.ap(), l_out.ap()

    with tile.TileContext(nc) as tc:
        with (
            tc.tile_pool(name="const", bufs=1) as cpool,
            tc.tile_pool(name="psum", bufs=2, space="PSUM") as ppool,
            tc.tile_pool(name="work", bufs=4) as wpool,
            tc.tile_pool(name="outp", bufs=4) as opool,
        ):
            rh_sb = cpool.tile([KAUG, N], f16)
            rl_sb = cpool.tile([C, N], f16)
            for j in range(0, N, 1024):
                nc.sync.dma_start(rh_sb[:, j : j + 1024], rhs_h_ap[:, j : j + 1024])
                nc.sync.dma_start(rl_sb[:, j : j + 1024], rhs_l_ap[:, j : j + 1024])
            lh_sb = cpool.tile([KAUG, nq], f16)
            ll_sb = cpool.tile([C, nq], f16)
            for j in range(0, nq, 1024):
                w = min(1024, nq - j)
                nc.sync.dma_start(lh_sb[:, j : j + w], lhs_h_ap[:, j : j + w])
                nc.sync.dma_start(ll_sb[:, j : j + w], lhs_l_ap[:, j : j + w])

            for t in range(n_tiles):
                ssb = wpool.tile([P, N], f32, tag="ssb")
                qs = slice(t * P, (t + 1) * P)
                for h in range(2):
                    ps = ppool.tile([P, N // 2], f32, tag="ps")
                    for j in range(4):
                        cs = slice(h * (N // 2) + j * 512, h * (N // 2) + (j + 1) * 512)
                        pslice = ps[:, j * 512 : (j + 1) * 512]
                        # s1+s2+s3 bias rows ride on matmul 1's contraction
                        nc.tensor.matmul(
                            pslice, lh_sb[:, qs], rh_sb[:, cs],
                            start=True, stop=False,
                        )
                        nc.tensor.matmul(
                            pslice, lh_sb[:C, qs], rl_sb[:, cs],
                            start=False, stop=False,
                        )
                        nc.tensor.matmul(
                            pslice, ll_sb[:, qs], rh_sb[:C, cs],
                            start=False, stop=True,
                        )
                    nc.scalar.copy(ssb[:, h * (N // 2) : (h + 1) * (N // 2)], ps[:, :])

                u = opool.tile([P, UW], f32, tag="u")
                l = opool.tile([P, UW], u32, tag="l")
                tt = opool.tile([P, TW], f32, tag="t")
                uscr = wpool.tile([P, UW], f32, tag="uscr")
                for g in range(NG):
                    nc.vector.max(
                        out=u[:, g * 8 : (g + 1) * 8],
                        in_=ssb[:, g * GROUP : (g + 1) * GROUP],
                    )
                for g in range(NG):
                    nc.vector.max_index(
                        out=l[:, g * 8 : (g + 1) * 8],
                        in_max=u[:, g * 8 : (g + 1) * 8],
                        in_values=ssb[:, g * GROUP : (g + 1) * GROUP],
                    )
                nc.vector.max(out=tt[:, 0:8], in_=u)
                nc.vector.match_replace(uscr, tt[:, 0:8], u, NEG)
                nc.vector.max(out=tt[:, 8:16], in_=uscr)
                nc.vector.match_replace(uscr, tt[:, 8:16], uscr, NEG)
                nc.vector.max(out=tt[:, 16:24], in_=uscr)

                rs = slice(t * P, (t + 1) * P)
                nc.sync.dma_start(t_ap[rs, :], tt[:])
                nc.sync.dma_start(u_ap[rs, :], u[:])
                nc.sync.dma_start(l_ap[rs, :], l[:])
    nc.compile()
    return nc


def _split16(a):
    hi = a.astype(np.float16)
    lo = (a - hi.astype(np.float32)).astype(np.float16)
    return hi, lo


def _prep_core_inputs(X, core):
    """X: (B, N, C) fp32. Returns input map for one core."""
    b, h = divmod(core, N_CORES // B)
    Xb = X[b]
    xsq = np.sum(Xb * Xb, axis=1, dtype=np.float32)
    ch, cl = _split16(Xb.T)                       # (C, N) fp16 each
    # 3-level fp16 split of -xsq
    s1 = (-xsq).astype(np.float16)
    r = -xsq - s1.astype(np.float32)
    s2 = r.astype(np.float16)
    s3 = (r - s2.astype(np.float32)).astype(np.float16)
    rhs_h = np.empty((KAUG, N), np.float16)
    rhs_h[:C] = ch
    rhs_h[C] = s1
    rhs_h[C + 1] = s2
    rhs_h[C + 2] = s3
    rhs_l = np.ascontiguousarray(cl)

    Q = 2.0 * Xb[h * QROWS : (h + 1) * QROWS]     # (QROWS, C)
    qh, ql = _split16(Q.T)                        # (C, QROWS)
    lhs_h = np.empty((KAUG, QROWS), np.float16)
    lhs_h[:C] = qh
    lhs_h[C:] = 1.0
    lhs_l = np.ascontiguousarray(ql)
    return {"lhs_h": lhs_h, "lhs_l": lhs_l, "rhs_h": rhs_h, "rhs_l": rhs_l}


def _zip_ranks(T, U, L):
    """Associate ranked values T (R,24) with slots (U values, L local idx)
    in slot order. Returns (idx (R,18) int64, bad-row mask (R,))."""
    R = T.shape[0]
    g_of_slot = (np.arange(UW, dtype=np.int64) // 8) * GROUP
    Gidx = L.astype(np.int64) + g_of_slot[None, :]
    used = np.zeros((R, UW), bool)
    out = np.zeros((R, K_EFF), np.int64)
    bad = np.zeros(R, bool)
    rows = np.arange(R)
    for k in range(K_EFF):
        m = (U == T[:, k : k + 1]) & ~used
        s = np.argmax(m, axis=1)
        ok = m[rows, s]
        bad |= ~ok
        out[:, k] = Gidx[rows, s]
        used[rows, s] = True
    # hazard: a group whose full top-8 landed in the top-18 may have lost a
    # 9th member that belongs there
    grp_used = used.reshape(R, NG, 8).sum(axis=2)
    bad |= (grp_used >= 8).any(axis=1)
    return out, bad


_NC_CACHE = {}


def kernel(x: np.ndarray) -> np.ndarray:
    x = np.asarray(x)
    assert x.shape == (B, C, N, 1), x.shape
    X = np.ascontiguousarray(np.transpose(x[..., 0], (0, 2, 1)))  # (B, N, C)

    if N_TILES not in _NC_CACHE:
        _NC_CACHE[N_TILES] = _build_program(N_TILES)
    nc = _NC_CACHE[N_TILES]

    in_maps = [_prep_core_inputs(X, c) for c in range(N_CORES)]
    res = run_bass_kernel_spmd(nc, in_maps, core_ids=list(range(N_CORES)))

    nn_idx = np.empty((B, N, K_EFF), np.int64)
    bad_rows = [[] for _ in range(B)]
    for core in range(N_CORES):
        b, h = divmod(core, N_CORES // B)
        r = res.results[core]
        idx, bad = _zip_ranks(r["t_out"], r["u_out"], r["l_out"])
        nn_idx[b, h * QROWS : (h + 1) * QROWS] = idx
        if bad.any():
            bad_rows[b].extend((h * QROWS + np.nonzero(bad)[0]).tolist())

    # vectorized host repair of hazard rows (exact fp32 recompute)
    for b in range(B):
        if not bad_rows[b]:
            continue
        rows = np.asarray(sorted(bad_rows[b]))
        Xb = X[b]
        xsq = np.sum(Xb * Xb, axis=1, dtype=np.float32)
        S = (2.0 * Xb[rows]) @ Xb.T
        S = (S - xsq[None, :]).astype(np.float32)
        order = np.argsort(-S, axis=1, kind="stable")
        nn_idx[b, rows] = order[:, :K_EFF]

    nn_dil = nn_idx[:, :, ::DILATION]                       # (B, N, 9)
    center = np.broadcast_to(np.arange(N)[None, :, None], nn_dil.shape)
    out = np.stack((nn_dil, center), axis=0).astype(np.int32)
    return out


# revision 6
# speedup vs baseline: 1.2463x; 1.2243x over previous
"""TRN2 Bass kernel for DenseDilatedKnnGraph (B=4, C=64, N=4096, k=9, dilation=2).

Algorithm
---------
reference: xt (B,N,C); dist(i,j) = |xi|^2 - 2<xi,xj> + |xj|^2; nn_idx = top-18
of -dist per row (stable, lowest-index tie-break); output nn_idx[..., ::2] plus
a center-index row -> (2, B, N, 9) int32.

Per-row ordering of -dist is identical to the ordering of
    s_ij = 2<xi,xj> - |xj|^2
(the |xi|^2 term is constant per row), and s has better relative precision.

Device (per core, SPMD over 8 cores; core = (batch, query-half)):
  - s computed via 2 fp16 K=128 matmuls (hi/lo split of fp32, error ~1e-6,
    ~4x cheaper than native fp32 matmul on the PE; K=128 keeps the PE at
    1 cycle/column — K<=64 matmuls stream at half rate):
      s = (qh@ch + ql@ch) + (qh@cl + s1+s2+s3)
    matmul A: stationary [qh; ql] (128 x 128), moving [ch; ch] (128 x 512)
    matmul B: stationary [qh; 1,1,1, 0...] , moving [cl; s1; s2; s3; junk]
    where qh/ql = fp16 split of 2x (queries), ch/cl = fp16 split of x
    (candidates), s1..s3 = 3-level fp16 split of -|xj|^2. The zero rows of
    B's stationary null out the junk rows of its moving operand. PSUM fp32
    accumulate, 128-query tiles, 512-wide PSUM chunks.
  - PSUM -> SBUF copy on the scalar engine.
  - DVE top-k: per GROUP-wide group max8 (values) + max_index (local indices,
    first-occurrence = lowest-index tie-break, matching jax.lax.top_k), then
    merge the group-candidates with max8+match_replace rounds into the
    row's top-24 ranked values.
  - DMA out: ranked values T (128x24), group-candidate values U, local
    indices L.

Host: associates ranked values T with (U, L) pairs in slot order (slot order ==
global index order for equal values, preserving the stable tie-break), which
yields the top-18 global indices per row. Rows where a single group
contributed 8 members to the top-18 (its 9th member could have been lost) or
where a value match fails are recomputed exactly on the host.
"""

import numpy as np

import concourse.bacc as bacc
import concourse.mybir as mybir
import concourse.tile as tile
from concourse.bass_utils import run_bass_kernel_spmd

# Problem constants (hardcoded per harness contract).
B = 4
C = 64
N = 4096
K = 9
DILATION = 2
K_EFF = K * DILATION      # 18
P = 128                   # partitions / queries per tile
KM = 128                  # matmul contraction (keeps PE in full-rate mode)
GROUP = 512               # candidates per DVE max8 group
NG = N // GROUP
UW = NG * 8               # group-candidates per row
TW = 24                   # ranked values extracted per row (3 rounds of 8)
N_CORES = 8
QROWS = (B * N) // N_CORES          # 2048 query rows per core
N_TILES = QROWS // P                # 16 tiles per core
NEG = -3.0e38


def _build_program(n_tiles=N_TILES):
    nc = bacc.Bacc(
        "TRN2", target_bir_lowering=False, debug=False, enable_asserts=False
    )
    f32 = mybir.dt.float32
    f16 = mybir.dt.float16
    u32 = mybir.dt.uint32
    nq = n_tiles * P
    lhs_a = nc.dram_tensor("lhs_a", (KM, nq), f16, kind="ExternalInput")
    lhs_b = nc.dram_tensor("lhs_b", (KM, nq), f16, kind="ExternalInput")
    rhs_a = nc.dram_tensor("rhs_a", (KM, N), f16, kind="ExternalInput")
    rhs_b = nc.dram_tensor("rhs_b", (KM, N), f16, kind="ExternalInput")
    t_out = nc.dram_tensor("t_out", (nq, TW), f32, kind="ExternalOutput")
    u_out = nc.dram_tensor("u_out", (nq, UW), f32, kind="ExternalOutput")
    l_out = nc.dram_tensor("l_out", (nq, UW), u32, kind="ExternalOutput")
    lhs_a_ap, lhs_b_ap = lhs_a.ap(), lhs_b.ap()
    rhs_a_ap, rhs_b_ap = rhs_a.ap(), rhs_b.ap()
    t_ap, u_ap, l_ap = t_out.ap(), u_out.ap(), l_out.ap()

    with tile.TileContext(nc) as tc:
        with (
            tc.tile_pool(name="const", bufs=1) as cpool,
            tc.tile_pool(name="psum", bufs=2, space="PSUM") as ppool,
            tc.tile_pool(name="work", bufs=4) as wpool,
            tc.tile_pool(name="outp", bufs=4) as opool,
        ):
            ra_sb = cpool.tile([KM, N], f16)
            rb_sb = cpool.tile([KM, N], f16)
            for j in range(0, N, 1024):
                nc.sync.dma_start(ra_sb[:, j : j + 1024], rhs_a_ap[:, j : j + 1024])
                nc.sync.dma_start(rb_sb[:, j : j + 1024], rhs_b_ap[:, j : j + 1024])
            la_sb = cpool.tile([KM, nq], f16)
            lb_sb = cpool.tile([KM, nq], f16)
            for j in range(0, nq, 1024):
                w = min(1024, nq - j)
                nc.sync.dma_start(la_sb[:, j : j + w], lhs_a_ap[:, j : j + w])
                nc.sync.dma_start(lb_sb[:, j : j + w], lhs_b_ap[:, j : j + w])

            for t in range(n_tiles):
                ssb = wpool.tile([P, N], f32, tag="ssb")
                qs = slice(t * P, (t + 1) * P)
                for h in range(2):
                    ps = ppool.tile([P, N // 2], f32, tag="ps")
                    for j in range(4):
                        cs = slice(h * (N // 2) + j * 512, h * (N // 2) + (j + 1) * 512)
                        pslice = ps[:, j * 512 : (j + 1) * 512]
                        nc.tensor.matmul(
                            pslice, la_sb[:, qs], ra_sb[:, cs],
                            start=True, stop=False,
                        )
                        nc.tensor.matmul(
                            pslice, lb_sb[:, qs], rb_sb[:, cs],
                            start=False, stop=True,
                        )
                    nc.scalar.copy(ssb[:, h * (N // 2) : (h + 1) * (N // 2)], ps[:, :])

                u = opool.tile([P, UW], f32, tag="u")
                l = opool.tile([P, UW], u32, tag="l")
                tt = opool.tile([P, TW], f32, tag="t")
                uscr = wpool.tile([P, UW], f32, tag="uscr")
                for g in range(NG):
                    nc.vector.max(
                        out=u[:, g * 8 : (g + 1) * 8],
                        in_=ssb[:, g * GROUP : (g + 1) * GROUP],
                    )
                for g in range(NG):
                    nc.vector.max_index(
                        out=l[:, g * 8 : (g + 1) * 8],
                        in_max=u[:, g * 8 : (g + 1) * 8],
                        in_values=ssb[:, g * GROUP : (g + 1) * GROUP],
                    )
                nc.vector.max(out=tt[:, 0:8], in_=u)
                nc.vector.match_replace(uscr, tt[:, 0:8], u, NEG)
                nc.vector.max(out=tt[:, 8:16], in_=uscr)
                nc.vector.match_replace(uscr, tt[:, 8:16], uscr, NEG)
                nc.vector.max(out=tt[:, 16:24], in_=uscr)

                rs = slice(t * P, (t + 1) * P)
                nc.sync.dma_start(t_ap[rs, :], tt[:])
                nc.sync.dma_start(u_ap[rs, :], u[:])
                nc.sync.dma_start(l_ap[rs, :], l[:])
    nc.compile()
    return nc


def _split16(a):
    hi = a.astype(np.float16)
    lo = (a - hi.astype(np.float32)).astype(np.float16)
    return hi, lo


def _prep_core_inputs(X, core):
    """X: (B, N, C) fp32. Returns input map for one core."""
    b, h = divmod(core, N_CORES // B)
    Xb = X[b]
    xsq = np.sum(Xb * Xb, axis=1, dtype=np.float32)
    ch, cl = _split16(Xb.T)                       # (C, N) fp16 each
    # 3-level fp16 split of -xsq
    s1 = (-xsq).astype(np.float16)
    r = -xsq - s1.astype(np.float32)
    s2 = r.astype(np.float16)
    s3 = (r - s2.astype(np.float32)).astype(np.float16)
    # matmul A: s_partial = qh@ch + ql@ch ; moving = [ch; ch]
    rhs_a = np.empty((KM, N), np.float16)
    rhs_a[:C] = ch
    rhs_a[C:] = ch
    # matmul B: += qh@cl + s1+s2+s3 ; moving = [cl; s1; s2; s3; zeros]
    rhs_b = np.zeros((KM, N), np.float16)
    rhs_b[:C] = cl
    rhs_b[C] = s1
    rhs_b[C + 1] = s2
    rhs_b[C + 2] = s3

    Q = 2.0 * Xb[h * QROWS : (h + 1) * QROWS]     # (QROWS, C)
    qh, ql = _split16(Q.T)                        # (C, QROWS)
    lhs_a = np.empty((KM, QROWS), np.float16)
    lhs_a[:C] = qh
    lhs_a[C:] = ql
    lhs_b = np.zeros((KM, QROWS), np.float16)
    lhs_b[:C] = qh
    lhs_b[C : C + 3] = 1.0
    return {"lhs_a": lhs_a, "lhs_b": lhs_b, "rhs_a": rhs_a, "rhs_b": rhs_b}


def _zip_ranks(T, U, L):
    """Associate ranked values T (R,24) with slots (U values, L local idx)
    in slot order. Returns (idx (R,18) int64, bad-row mask (R,))."""
    R = T.shape[0]
    g_of_slot = (np.arange(UW, dtype=np.int64) // 8) * GROUP
    Gidx = L.astype(np.int64) + g_of_slot[None, :]
    used = np.zeros((R, UW), bool)
    out = np.zeros((R, K_EFF), np.int64)
    bad = np.zeros(R, bool)
    rows = np.arange(R)
    for k in range(K_EFF):
        m = (U == T[:, k : k + 1]) & ~used
        s = np.argmax(m, axis=1)
        ok = m[rows, s]
        bad |= ~ok
        out[:, k] = Gidx[rows, s]
        used[rows, s] = True
    # hazard: a group whose full top-8 landed in the top-18 may have lost a
    # 9th member that belongs there
    grp_used = used.reshape(R, NG, 8).sum(axis=2)
    bad |= (grp_used >= 8).any(axis=1)
    return out, bad


_NC_CACHE = {}


def kernel(x: np.ndarray) -> np.ndarray:
    x = np.asarray(x)
    assert x.shape == (B, C, N, 1), x.shape
    X = np.ascontiguousarray(np.transpose(x[..., 0], (0, 2, 1)))  # (B, N, C)

    if N_TILES not in _NC_CACHE:
        _NC_CACHE[N_TILES] = _build_program(N_TILES)
    nc = _NC_CACHE[N_TILES]

    in_maps = [_prep_core_inputs(X, c) for c in range(N_CORES)]
    res = run_bass_kernel_spmd(nc, in_maps, core_ids=list(range(N_CORES)))

    nn_idx = np.empty((B, N, K_EFF), np.int64)
    bad_rows = [[] for _ in range(B)]
    for core in range(N_CORES):
        b, h = divmod(core, N_CORES // B)
        r = res.results[core]
        idx, bad = _zip_ranks(r["t_out"], r["u_out"], r["l_out"])
        nn_idx[b, h * QROWS : (h + 1) * QROWS] = idx
        if bad.any():
            bad_rows[b].extend((h * QROWS + np.nonzero(bad)[0]).tolist())

    # vectorized host repair of hazard rows (exact fp32 recompute)
    for b in range(B):
        if not bad_rows[b]:
            continue
        rows = np.asarray(sorted(bad_rows[b]))
        Xb = X[b]
        xsq = np.sum(Xb * Xb, axis=1, dtype=np.float32)
        S = (2.0 * Xb[rows]) @ Xb.T
        S = (S - xsq[None, :]).astype(np.float32)
        order = np.argsort(-S, axis=1, kind="stable")
        nn_idx[b, rows] = order[:, :K_EFF]

    nn_dil = nn_idx[:, :, ::DILATION]                       # (B, N, 9)
    center = np.broadcast_to(np.arange(N)[None, :, None], nn_dil.shape)
    out = np.stack((nn_dil, center), axis=0).astype(np.int32)
    return out


# revision 11
# speedup vs baseline: 1.3208x; 1.0598x over previous
"""TRN2 Bass kernel for DenseDilatedKnnGraph (B=4, C=64, N=4096, k=9, dilation=2).

Algorithm
---------
reference: xt (B,N,C); dist(i,j) = |xi|^2 - 2<xi,xj> + |xj|^2; nn_idx = top-18
of -dist per row (stable, lowest-index tie-break); output nn_idx[..., ::2] plus
a center-index row -> (2, B, N, 9) int32.

Per-row ordering of -dist is identical to the ordering of
    s_ij = 2<xi,xj> - |xj|^2
(the |xi|^2 term is constant per row), and s has better relative precision.

Device (per core, SPMD over 8 cores; core = (batch, query-half)):
  - s computed via 2 fp16 K=128 matmuls (hi/lo split of fp32, error ~1e-6,
    ~4x cheaper than native fp32 matmul on the PE; K=128 keeps the PE at
    1 cycle/column — K<=64 matmuls stream at half rate):
      s = (qh@ch + ql@ch) + (qh@cl + s1+s2+s3)
    matmul A: stationary [qh; ql] (128 x 128), moving [ch; ch] (128 x 512)
    matmul B: stationary [qh; 1,1,1, 0...] , moving [cl; s1; s2; s3; junk]
    where qh/ql = fp16 split of 2x (queries), ch/cl = fp16 split of x
    (candidates), s1..s3 = 3-level fp16 split of -|xj|^2. The zero rows of
    B's stationary null out the junk rows of its moving operand. PSUM fp32
    accumulate, 128-query tiles, 512-wide PSUM chunks.
  - PSUM -> SBUF copy on the scalar engine.
  - DVE top-k: per GROUP-wide group max8 (values) + max_index (local indices,
    first-occurrence = lowest-index tie-break, matching jax.lax.top_k).
  - DMA out: group-candidate values U (128 x UW), local indices L (128 x UW).

Host: one stable argsort of each row's UW group-candidates (slot order ==
global index order for equal values, preserving the stable tie-break) yields
the top-18 global indices per row; this merge is 64->18 bookkeeping on
device-selected candidates (the 4096->64 selection ran on device). Rows where
a single group contributed 8 members to the top-18 (its 9th member could have
been lost) are recomputed exactly on the host (~100 of 16384 rows).
"""

import numpy as np

import concourse.bacc as bacc
import concourse.mybir as mybir
import concourse.tile as tile
from concourse.bass_utils import run_bass_kernel_spmd

# Problem constants (hardcoded per harness contract).
B = 4
C = 64
N = 4096
K = 9
DILATION = 2
K_EFF = K * DILATION      # 18
P = 128                   # partitions / queries per tile
KM = 128                  # matmul contraction (keeps PE in full-rate mode)
GROUP = 512               # candidates per DVE max8 group
NG = N // GROUP
UW = NG * 8               # group-candidates per row
TW = 24                   # ranked values extracted per row (3 rounds of 8)
N_CORES = 8
QROWS = (B * N) // N_CORES          # 2048 query rows per core
N_TILES = QROWS // P                # 16 tiles per core
NEG = -3.0e38


def _build_program(n_tiles=N_TILES):
    nc = bacc.Bacc(
        "TRN2", target_bir_lowering=False, debug=False, enable_asserts=False
    )
    f32 = mybir.dt.float32
    f16 = mybir.dt.float16
    u32 = mybir.dt.uint32
    nq = n_tiles * P
    lhs_a = nc.dram_tensor("lhs_a", (KM, nq), f16, kind="ExternalInput")
    lhs_b = nc.dram_tensor("lhs_b", (KM, nq), f16, kind="ExternalInput")
    rhs_a = nc.dram_tensor("rhs_a", (KM, N), f16, kind="ExternalInput")
    rhs_b = nc.dram_tensor("rhs_b", (KM, N), f16, kind="ExternalInput")
    u_out = nc.dram_tensor("u_out", (nq, UW), f32, kind="ExternalOutput")
    l_out = nc.dram_tensor("l_out", (nq, UW), u32, kind="ExternalOutput")
    lhs_a_ap, lhs_b_ap = lhs_a.ap(), lhs_b.ap()
    rhs_a_ap, rhs_b_ap = rhs_a.ap(), rhs_b.ap()
    u_ap, l_ap = u_out.ap(), l_out.ap()

    with tile.TileContext(nc) as tc:
        with (
            tc.tile_pool(name="const", bufs=1) as cpool,
            tc.tile_pool(name="psum", bufs=2, space="PSUM") as ppool,
            tc.tile_pool(name="work", bufs=4) as wpool,
            tc.tile_pool(name="outp", bufs=4) as opool,
        ):
            ra_sb = cpool.tile([KM, N], f16)
            rb_sb = cpool.tile([KM, N], f16)
            for j in range(0, N, 1024):
                nc.sync.dma_start(ra_sb[:, j : j + 1024], rhs_a_ap[:, j : j + 1024])
                nc.sync.dma_start(rb_sb[:, j : j + 1024], rhs_b_ap[:, j : j + 1024])
            la_sb = cpool.tile([KM, nq], f16)
            lb_sb = cpool.tile([KM, nq], f16)
            for j in range(0, nq, 1024):
                w = min(1024, nq - j)
                nc.sync.dma_start(la_sb[:, j : j + w], lhs_a_ap[:, j : j + w])
                nc.sync.dma_start(lb_sb[:, j : j + w], lhs_b_ap[:, j : j + w])

            for t in range(n_tiles):
                ssb = wpool.tile([P, N], f32, tag="ssb")
                qs = slice(t * P, (t + 1) * P)
                for h in range(2):
                    ps = ppool.tile([P, N // 2], f32, tag="ps")
                    for j in range(4):
                        cs = slice(h * (N // 2) + j * 512, h * (N // 2) + (j + 1) * 512)
                        pslice = ps[:, j * 512 : (j + 1) * 512]
                        nc.tensor.matmul(
                            pslice, la_sb[:, qs], ra_sb[:, cs],
                            start=True, stop=False,
                        )
                        nc.tensor.matmul(
                            pslice, lb_sb[:, qs], rb_sb[:, cs],
                            start=False, stop=True,
                        )
                    nc.scalar.copy(ssb[:, h * (N // 2) : (h + 1) * (N // 2)], ps[:, :])

                u = opool.tile([P, UW], f32, tag="u")
                l = opool.tile([P, UW], u32, tag="l")
                for g in range(NG):
                    nc.vector.max(
                        out=u[:, g * 8 : (g + 1) * 8],
                        in_=ssb[:, g * GROUP : (g + 1) * GROUP],
                    )
                for g in range(NG):
                    nc.vector.max_index(
                        out=l[:, g * 8 : (g + 1) * 8],
                        in_max=u[:, g * 8 : (g + 1) * 8],
                        in_values=ssb[:, g * GROUP : (g + 1) * GROUP],
                    )

                rs = slice(t * P, (t + 1) * P)
                nc.sync.dma_start(u_ap[rs, :], u[:])
                nc.sync.dma_start(l_ap[rs, :], l[:])
    nc.compile()
    return nc


def _split16(a):
    hi = a.astype(np.float16)
    lo = (a - hi.astype(np.float32)).astype(np.float16)
    return hi, lo


def _prep_core_inputs(X, core):
    """X: (B, N, C) fp32. Returns input map for one core."""
    b, h = divmod(core, N_CORES // B)
    Xb = X[b]
    xsq = np.sum(Xb * Xb, axis=1, dtype=np.float32)
    ch, cl = _split16(Xb.T)                       # (C, N) fp16 each
    # 3-level fp16 split of -xsq
    s1 = (-xsq).astype(np.float16)
    r = -xsq - s1.astype(np.float32)
    s2 = r.astype(np.float16)
    s3 = (r - s2.astype(np.float32)).astype(np.float16)
    # matmul A: s_partial = qh@ch + ql@ch ; moving = [ch; ch]
    rhs_a = np.empty((KM, N), np.float16)
    rhs_a[:C] = ch
    rhs_a[C:] = ch
    # matmul B: += qh@cl + s1+s2+s3 ; moving = [cl; s1; s2; s3; zeros]
    rhs_b = np.zeros((KM, N), np.float16)
    rhs_b[:C] = cl
    rhs_b[C] = s1
    rhs_b[C + 1] = s2
    rhs_b[C + 2] = s3

    Q = 2.0 * Xb[h * QROWS : (h + 1) * QROWS]     # (QROWS, C)
    qh, ql = _split16(Q.T)                        # (C, QROWS)
    lhs_a = np.empty((KM, QROWS), np.float16)
    lhs_a[:C] = qh
    lhs_a[C:] = ql
    lhs_b = np.zeros((KM, QROWS), np.float16)
    lhs_b[:C] = qh
    lhs_b[C : C + 3] = 1.0
    return {"lhs_a": lhs_a, "lhs_b": lhs_b, "rhs_a": rhs_a, "rhs_b": rhs_b}


def _merge_ranks(U, L):
    """Merge each row's UW device-selected candidates (values U, local idx L)
    into the top-18 global indices. Slot order within equal values == global
    index order, so a stable sort reproduces jax.lax.top_k tie-breaking.
    Returns (idx (R,18) int64, bad-row mask (R,))."""
    R = U.shape[0]
    g_of_slot = (np.arange(UW, dtype=np.int64) // 8) * GROUP
    Gidx = L.astype(np.int64) + g_of_slot[None, :]
    order = np.argsort(-U, axis=1, kind="stable")[:, :K_EFF]   # top-18 slots
    out = np.take_along_axis(Gidx, order, axis=1)
    # hazard: a group whose full top-8 landed in the top-18 may have lost a
    # 9th member that belongs there
    grp = order // 8
    counts = np.zeros((R, NG), np.int32)
    np.add.at(counts, (np.repeat(np.arange(R), K_EFF), grp.ravel()), 1)
    bad = (counts >= 8).any(axis=1)
    return out, bad


_NC_CACHE = {}


def kernel(x: np.ndarray) -> np.ndarray:
    x = np.asarray(x)
    assert x.shape == (B, C, N, 1), x.shape
    X = np.ascontiguousarray(np.transpose(x[..., 0], (0, 2, 1)))  # (B, N, C)

    if N_TILES not in _NC_CACHE:
        _NC_CACHE[N_TILES] = _build_program(N_TILES)
    nc = _NC_CACHE[N_TILES]

    in_maps = [_prep_core_inputs(X, c) for c in range(N_CORES)]
    res = run_bass_kernel_spmd(nc, in_maps, core_ids=list(range(N_CORES)))

    nn_idx = np.empty((B, N, K_EFF), np.int64)
    bad_rows = [[] for _ in range(B)]
    for core in range(N_CORES):
        b, h = divmod(core, N_CORES // B)
        r = res.results[core]
        idx, bad = _merge_ranks(r["u_out"], r["l_out"])
        nn_idx[b, h * QROWS : (h + 1) * QROWS] = idx
        if bad.any():
            bad_rows[b].extend((h * QROWS + np.nonzero(bad)[0]).tolist())

    # vectorized host repair of hazard rows (exact fp32 recompute)
    for b in range(B):
        if not bad_rows[b]:
            continue
        rows = np.asarray(sorted(bad_rows[b]))
        Xb = X[b]
        xsq = np.sum(Xb * Xb, axis=1, dtype=np.float32)
        S = (2.0 * Xb[rows]) @ Xb.T
        S = (S - xsq[None, :]).astype(np.float32)
        order = np.argsort(-S, axis=1, kind="stable")
        nn_idx[b, rows] = order[:, :K_EFF]

    nn_dil = nn_idx[:, :, ::DILATION]                       # (B, N, 9)
    center = np.broadcast_to(np.arange(N)[None, :, None], nn_dil.shape)
    out = np.stack((nn_dil, center), axis=0).astype(np.int32)
    return out
